# revision 14
# baseline (speedup 1.0000x reference)
"""Trainium2 kernel for nn_BeliefStateTracker (particle-filter belief update).

Data-parallel over the batch dim: the 32 batch rows are sharded 4-per-core
across the 8 NeuronCores via jax.pmap (every op in update() is independent
per batch row, so no cross-device communication is needed). The whole update
is one fused XLA program per core compiled by neuronx-cc for trn2.

Note: a hand-written Bass/Tile version of this kernel (feature-major layout,
weight-stationary matmuls, folded action/observation biases) is staged in
kernel_bass_wip.py.bak; the container's walrus build rejects any instruction
carrying more than one semaphore wait ("Too many sync wait commands",
CoreV2GenImpl.cpp setupSyncWait), which the Tile scheduler emits freely, so
every Tile-generated NEFF fails codegen here. The XLA path compiles and runs
on the same 8 NeuronCores.
"""

import numpy as np

B, N, H, A, O = 32, 1024, 256, 32, 256
NCORES = 8
RPC = B // NCORES
RESAMPLE_THRESHOLD = 0.5
EPS = 1e-10

_CACHE = {}


def _update(particles, weights, action, observation, u,
            Wr, br, Wu, bu, Wc, bc, W0, b0, W1, b1, W2, b2):
    import jax
    import jax.numpy as jnp

    b, n, h = particles.shape
    act = jnp.broadcast_to(action[:, None, :], (b, n, action.shape[-1]))
    obs = jnp.broadcast_to(observation[:, None, :], (b, n, observation.shape[-1]))

    x = jnp.concatenate([particles, act], axis=-1)
    reset = jax.nn.sigmoid(x @ Wr + br)
    update = jax.nn.sigmoid(x @ Wu + bu)
    x_reset = jnp.concatenate([particles * reset, act], axis=-1)
    cand = jnp.tanh(x_reset @ Wc + bc)
    nxt = (1.0 - update) * particles + update * cand

    z = jnp.concatenate([nxt, obs], axis=-1)
    z = jax.nn.gelu(z @ W0 + b0, approximate=False)
    z = jax.nn.gelu(z @ W1 + b1, approximate=False)
    log_lik = (z @ W2 + b2)[..., 0]

    log_w = jnp.log(weights + EPS) + log_lik
    log_w = log_w - jnp.max(log_w, axis=1, keepdims=True)
    new_w = jax.nn.softmax(log_w, axis=1)

    ess = 1.0 / (jnp.sum(new_w * new_w, axis=1) + EPS)
    should = ess < RESAMPLE_THRESHOLD * n

    def _resample():
        csum = jnp.cumsum(new_w, axis=1)
        pos = u / n + jnp.arange(n, dtype=jnp.float32) / n
        pos = jnp.minimum(pos, 0.9999)
        # searchsorted(csum, pos) == #{m : csum[m] < pos[n]}. vmap(searchsorted)
        # lowers to a while-loop; use the count formulation blocked over m so
        # it stays elementwise/reduce shaped.
        idx = jnp.zeros(pos.shape, jnp.int32)
        BLK = 256
        for m0 in range(0, n, BLK):
            blk = csum[:, m0:m0 + BLK]
            idx = idx + jnp.sum(
                (blk[:, :, None] < pos[:, None, :]).astype(jnp.int32), axis=1)
        idx = jnp.clip(idx, 0, n - 1)
        resampled = jnp.take_along_axis(nxt, idx[..., None], axis=1)
        uniform_w = jnp.full_like(new_w, 1.0 / n)
        nxt2 = jnp.where(should[:, None, None], resampled, nxt)
        nw2 = jnp.where(should[:, None], uniform_w, new_w)
        return nxt2, nw2

    # The gather + O(N^2) rank counts only matter when some row's ESS drops
    # below threshold; lax.cond skips the whole block at runtime otherwise.
    # (The axon jax patch restricts cond to (pred, true_fn, false_fn), so the
    # branches close over their operands.)
    nxt, new_w = jax.lax.cond(
        jnp.any(should), _resample, lambda: (nxt, new_w))

    belief = jnp.sum(nxt * new_w[..., None], axis=1)
    return belief, nxt, new_w, ess


def _get_fn():
    if "fn" not in _CACHE:
        import jax
        devs = jax.devices()[:NCORES]
        _CACHE["fn"] = jax.pmap(
            _update,
            in_axes=(0, 0, 0, 0, 0) + (None,) * 12,
            devices=devs,
        )
    return _CACHE["fn"]


def kernel(particles, weights, action, observation, u,
           Wr, br, Wu, bu, Wc, bc, W0, b0, W1, b1, W2, b2):
    f32 = np.float32
    particles = np.asarray(particles, f32).reshape(NCORES, RPC, N, H)
    weights = np.asarray(weights, f32).reshape(NCORES, RPC, N)
    action = np.asarray(action, f32).reshape(NCORES, RPC, A)
    observation = np.asarray(observation, f32).reshape(NCORES, RPC, O)
    u = np.asarray(u, f32).reshape(NCORES, RPC, 1)

    fn = _get_fn()
    belief, nxt, new_w, ess = fn(
        particles, weights, action, observation, u,
        np.asarray(Wr, f32), np.asarray(br, f32),
        np.asarray(Wu, f32), np.asarray(bu, f32),
        np.asarray(Wc, f32), np.asarray(bc, f32),
        np.asarray(W0, f32), np.asarray(b0, f32),
        np.asarray(W1, f32), np.asarray(b1, f32),
        np.asarray(W2, f32), np.asarray(b2, f32))

    belief = np.asarray(belief).reshape(B, H)
    nxt = np.asarray(nxt).reshape(B, N, H)
    new_w = np.asarray(new_w).reshape(B, N)
    ess = np.asarray(ess).reshape(B)
    return belief, nxt, new_w, ess


# revision 16
# speedup vs baseline: 1.1299x; 1.1299x over previous
"""Trainium2 kernel for nn_BeliefStateTracker (particle-filter belief update).

Data-parallel over the batch dim: the 32 batch rows are sharded 4-per-core
across the 8 NeuronCores via jax.pmap (every op in update() is independent
per batch row, so no cross-device communication is needed). The whole update
is one fused XLA program per core compiled by neuronx-cc for trn2.

Note: a hand-written Bass/Tile version of this kernel (feature-major layout,
weight-stationary matmuls, folded action/observation biases) is staged in
kernel_bass_wip.py.bak; the container's walrus build rejects any instruction
carrying more than one semaphore wait ("Too many sync wait commands",
CoreV2GenImpl.cpp setupSyncWait), which the Tile scheduler emits freely, so
every Tile-generated NEFF fails codegen here. The XLA path compiles and runs
on the same 8 NeuronCores.
"""

import numpy as np

B, N, H, A, O = 32, 1024, 256, 32, 256
NCORES = 8
RPC = B // NCORES
RESAMPLE_THRESHOLD = 0.5
EPS = 1e-10

_CACHE = {}


def _update(particles, weights, action, observation, u,
            Wr, br, Wu, bu, Wc, bc, W0, b0, W1, b1, W2, b2):
    import jax
    import jax.numpy as jnp

    b, n, h = particles.shape
    act = jnp.broadcast_to(action[:, None, :], (b, n, action.shape[-1]))
    obs = jnp.broadcast_to(observation[:, None, :], (b, n, observation.shape[-1]))

    x = jnp.concatenate([particles, act], axis=-1)
    reset = jax.nn.sigmoid(x @ Wr + br)
    update = jax.nn.sigmoid(x @ Wu + bu)
    x_reset = jnp.concatenate([particles * reset, act], axis=-1)
    cand = jnp.tanh(x_reset @ Wc + bc)
    nxt = (1.0 - update) * particles + update * cand

    z = jnp.concatenate([nxt, obs], axis=-1)
    z = jax.nn.gelu(z @ W0 + b0, approximate=False)
    z = jax.nn.gelu(z @ W1 + b1, approximate=False)
    log_lik = (z @ W2 + b2)[..., 0]

    log_w = jnp.log(weights + EPS) + log_lik
    log_w = log_w - jnp.max(log_w, axis=1, keepdims=True)
    new_w = jax.nn.softmax(log_w, axis=1)

    ess = 1.0 / (jnp.sum(new_w * new_w, axis=1) + EPS)
    should = ess < RESAMPLE_THRESHOLD * n

    def _resample():
        csum = jnp.cumsum(new_w, axis=1)
        pos = u / n + jnp.arange(n, dtype=jnp.float32) / n
        pos = jnp.minimum(pos, 0.9999)
        # searchsorted(csum, pos) == #{m : csum[m] < pos[n]}. vmap(searchsorted)
        # lowers to a while-loop; use the count formulation blocked over m so
        # it stays elementwise/reduce shaped.
        idx = jnp.zeros(pos.shape, jnp.int32)
        BLK = 256
        for m0 in range(0, n, BLK):
            blk = csum[:, m0:m0 + BLK]
            idx = idx + jnp.sum(
                (blk[:, :, None] < pos[:, None, :]).astype(jnp.int32), axis=1)
        idx = jnp.clip(idx, 0, n - 1)
        resampled = jnp.take_along_axis(nxt, idx[..., None], axis=1)
        uniform_w = jnp.full_like(new_w, 1.0 / n)
        nxt2 = jnp.where(should[:, None, None], resampled, nxt)
        nw2 = jnp.where(should[:, None], uniform_w, new_w)
        return nxt2, nw2

    # The gather + O(N^2) rank counts only matter when some row's ESS drops
    # below threshold; lax.cond skips the whole block at runtime otherwise.
    # (The axon jax patch restricts cond to (pred, true_fn, false_fn), so the
    # branches close over their operands.)
    nxt, new_w = jax.lax.cond(
        jnp.any(should), _resample, lambda: (nxt, new_w))

    belief = jnp.sum(nxt * new_w[..., None], axis=1)
    return belief, nxt, new_w, ess


def _get_fn():
    if "fn" not in _CACHE:
        import jax
        devs = jax.devices()[:NCORES]
        _CACHE["fn"] = jax.pmap(
            _update,
            in_axes=(0, 0, 0, 0, 0) + (None,) * 12,
            devices=devs,
        )
    return _CACHE["fn"]


def kernel(particles, weights, action, observation, u,
           Wr, br, Wu, bu, Wc, bc, W0, b0, W1, b1, W2, b2):
    f32 = np.float32
    particles = np.asarray(particles, f32).reshape(NCORES, RPC, N, H)
    weights = np.asarray(weights, f32).reshape(NCORES, RPC, N)
    action = np.asarray(action, f32).reshape(NCORES, RPC, A)
    observation = np.asarray(observation, f32).reshape(NCORES, RPC, O)
    u = np.asarray(u, f32).reshape(NCORES, RPC, 1)

    fn = _get_fn()
    belief, nxt, new_w, ess = fn(
        particles, weights, action, observation, u,
        np.asarray(Wr, f32), np.asarray(br, f32),
        np.asarray(Wu, f32), np.asarray(bu, f32),
        np.asarray(Wc, f32), np.asarray(bc, f32),
        np.asarray(W0, f32), np.asarray(b0, f32),
        np.asarray(W1, f32), np.asarray(b1, f32),
        np.asarray(W2, f32), np.asarray(b2, f32))

    import jax
    belief, nxt, new_w, ess = jax.device_get((belief, nxt, new_w, ess))
    return (np.asarray(belief).reshape(B, H),
            np.asarray(nxt).reshape(B, N, H),
            np.asarray(new_w).reshape(B, N),
            np.asarray(ess).reshape(B))
